# revision 47
# baseline (speedup 1.0000x reference)
"""Additive-attention pooling kernel for Trainium2 (8 NeuronCores, data-parallel).

Reference computation (per batch b):
    h      = tanh(x @ W1.T + b1)          # [S, D]
    scores = h @ w2 + b2                  # [S]
    w      = softmax(scores)              # [S]   (b2 cancels in softmax)
    ctx    = sum_s w[s] * x[s, :]         # [D]

Sharding: batch dim (64) split across 8 cores -> 8 batches/core.

Device layout: x is passed HOST-TRANSPOSED per batch as xT [D, S] in bf16 so
the main matmul streams xT directly (contraction dim d on PE partitions),
producing hT [e, s] tiles.  The score dot (w2 . h) runs as M=1 matmuls packed
two-at-a-time into disjoint PE column groups (tile_position) so they execute
concurrently.  exp(+denominator) ride the scalar engine's activation
accumulator; the weighted sum (ctx) is a fused multiply+reduce
(scalar_tensor_tensor) on the vector engine over the resident xT tiles
against gpsimd-broadcast exp rows.  The final divide-by-denominator is
host-side (0.0002% of the FLOPs).

The s-axis is processed in per-batch units (two 1024-halves; the very last
batch ends with two 512-quarters so its softmax/ctx tail is short), with the
score/ctx chain of unit u-1 emitted in the middle of unit u's matmul stream.
"""

import sys

try:
    import concourse  # noqa: F401  (resolves via the container's site config)
except ImportError:
    sys.path.insert(0, "/opt/trn_rl_repo")

from contextlib import ExitStack

import ml_dtypes
import numpy as np

B, S, D = 64, 2048, 512
NCORES = 8
BLOC = B // NCORES  # 8 batches per core
G = 4  # e-chunks of 128 (hidden dim)
C = 4  # d-chunks of 128 (input dim)
HALF = 1024

_CACHE = {}
TUNE = {
    "hook_g": 0,
    "xbufs": 5,
    "hbufs": 2,
    "coltile": True,
    "quarter_tail": True,
    "psh_bufs": 3,
    "split_xt_dma": False,
    "warmup_mms": 32,
}
NSLOT = 5  # max score-chunk (ctx/den) slots per batch


def _units(reps):
    """(batch, s_lo, width) units; last batch of the last rep ends in quarters."""
    units = []
    for r in range(reps):
        for b in range(BLOC):
            last = TUNE["quarter_tail"] and r == reps - 1 and b == BLOC - 1
            if last:
                units.append((b, 0, HALF))
                units.append((b, HALF, 512))
                units.append((b, HALF + 512, 512))
            else:
                units.append((b, 0, HALF))
                units.append((b, HALF, HALF))
    return units


def _build(reps=1):
    import concourse.tile as tile
    from concourse import bacc, mybir

    f32 = mybir.dt.float32
    bf16 = mybir.dt.bfloat16
    AF = mybir.ActivationFunctionType
    ALU = mybir.AluOpType

    nc = bacc.Bacc("TRN2", target_bir_lowering=False, debug=False, num_devices=NCORES)

    xt = nc.dram_tensor("xt", [BLOC, D, S], bf16, kind="ExternalInput").ap()
    # W1.T pre-arranged host-side into the exact SBUF layout [p, c, g, e]
    w1t = nc.dram_tensor("w1t", [128, C, G, 128], bf16, kind="ExternalInput").ap()
    b1r = nc.dram_tensor("b1r", [128, G], f32, kind="ExternalInput").ap()
    w2r = nc.dram_tensor("w2r", [128, G], bf16, kind="ExternalInput").ap()
    # raw outputs; the trivial final normalization (divide by softmax
    # denominator) happens host-side
    ctxuo = nc.dram_tensor(
        "ctxuo", [BLOC, 128, C, NSLOT], f32, kind="ExternalOutput"
    ).ap()
    expo = nc.dram_tensor("expo", [BLOC, S], bf16, kind="ExternalOutput").ap()
    den4o = nc.dram_tensor("den4o", [BLOC, NSLOT], f32, kind="ExternalOutput").ap()

    with tile.TileContext(nc) as tc, ExitStack() as ctx:
        const = ctx.enter_context(tc.tile_pool(name="const", bufs=1))
        xpool = ctx.enter_context(tc.tile_pool(name="x", bufs=TUNE["xbufs"]))
        hpool = ctx.enter_context(tc.tile_pool(name="h", bufs=TUNE["hbufs"]))
        spool = ctx.enter_context(tc.tile_pool(name="small", bufs=3))
        bpool = ctx.enter_context(tc.tile_pool(name="big", bufs=3))
        psum_h = ctx.enter_context(
            tc.tile_pool(name="psh", bufs=TUNE["psh_bufs"], space="PSUM")
        )
        psum_s = ctx.enter_context(tc.tile_pool(name="pss", bufs=2, space="PSUM"))

        # weights: chunk c=0 first (on the scalar-engine HWDGE ring) so the
        # first matmuls start as soon as it lands
        w1t_t = const.tile([128, C, G, 128], bf16)
        for c in range(C):
            nc.scalar.dma_start(w1t_t[:, c], w1t[:, c])
        b1_t = const.tile([128, G], f32)
        nc.gpsimd.dma_start(b1_t[:], b1r[:, :])
        w2_t = const.tile([128, G], bf16)
        nc.gpsimd.dma_start(w2_t[:], w2r[:, :])

        # PE HAM pre-warm: dummy matmuls on zeros while the first input DMAs
        # are in flight, so the real matmul stream starts at the 2.4 GHz
        # (warm) clock instead of paying the ~3.4us cold window
        n_warm = TUNE["warmup_mms"]
        if n_warm:
            wz = const.tile([128, 128], bf16, name="warm_z")
            nc.vector.memset(wz[:], 0.0)
            wp = psum_s.tile([64, 512], f32, tag="scps", name="warm_ps")
            for i in range(n_warm):
                nc.tensor.matmul(
                    wp[0:64, 0:128], lhsT=wz[:, 0:64], rhs=wz[:], start=True, stop=True
                )

        units = _units(reps)
        state = {}  # unit index -> dict
        bstate = {}  # rolling per-batch state (ctxu tile, slots)

        def emit_main(u, mid_hook=None):
            b, lo, W = units[u]
            st = state[u] = {"b": b, "lo": lo, "W": W}
            cw = min(512, W)
            nq = W // cw
            xt_t = xpool.tile([128, C, W], bf16, tag="xt", name=f"xt_{u}")
            xt_src = xt[b].rearrange("(c p) s -> p c s", p=128)[:, :, lo : lo + W]
            if u == 0:
                for c in range(C):
                    nc.sync.dma_start(xt_t[:, c], xt_src[:, c])
            elif TUNE["split_xt_dma"]:
                nc.sync.dma_start(xt_t[:, 0:2], xt_src[:, 0:2])
                nc.sync.dma_start(xt_t[:, 2:4], xt_src[:, 2:4])
            else:
                nc.sync.dma_start(xt_t[:], xt_src)
            st["xt"] = xt_t
            if lo == 0:
                bstate[b] = {
                    "ctxu": spool.tile(
                        [128, C, NSLOT], f32, tag="ctxu", name=f"ctxu_{u}"
                    ),
                    "slots": [],
                }
            st["bs"] = bstate[b]
            hs = []
            for g in range(G):
                hp = psum_h.tile([128, W], f32, tag="hps", name=f"hp_{u}_{g}")
                for c in range(C):
                    for q in range(nq):
                        nc.tensor.matmul(
                            hp[:, q * cw : (q + 1) * cw],
                            lhsT=w1t_t[:, c, g, :],
                            rhs=xt_t[:, c, q * cw : (q + 1) * cw],
                            start=(c == 0),
                            stop=(c == C - 1),
                        )
                h_g = hpool.tile([128, W], bf16, tag=f"h{g}", name=f"h{g}_{u}")
                nc.scalar.activation(
                    h_g[:], hp[:], AF.Tanh, bias=b1_t[:, g : g + 1], scale=1.0
                )
                hs.append(h_g)
                if g == TUNE["hook_g"] and mid_hook is not None:
                    mid_hook()
            st["hs"] = hs

        def emit_ctx_part(u, lo, width, slot, exp_ap):
            """Broadcast unnormalized exp and accumulate ctx via fused STT."""
            st = state[u]
            bs = st["bs"]
            ebc = bpool.tile([128, width], bf16, tag=f"ebc{width}", name=f"ebc_{u}_{slot}")
            nc.gpsimd.partition_broadcast(ebc[:], exp_ap)
            rel = lo - st["lo"]
            for c in range(C):
                scr = bpool.tile(
                    [128, width], bf16, tag=f"scr{c % 2}_{width}",
                    name=f"scr_{u}_{slot}_{c}",
                )
                nc.vector.scalar_tensor_tensor(
                    out=scr[:],
                    in0=st["xt"][:, c, rel : rel + width],
                    scalar=1.0,
                    in1=ebc[:],
                    op0=ALU.mult,
                    op1=ALU.mult,
                    accum_out=bs["ctxu"][:, c, slot : slot + 1],
                )
            bs["slots"].append(slot)

        def emit_scores_ctx(u):
            st = state[u]
            b, lo, W = st["b"], st["lo"], st["W"]
            cw = min(512, W)  # score-chunk width
            nq = W // cw
            coltile = TUNE["coltile"] and nq == 2
            if lo + W == S and b == BLOC - 1:
                # stage the already-complete ctx slots now, before this unit's
                # output DMAs can block the sync ring FIFO
                ps = len(st["bs"]["slots"])
                nc.sync.dma_start(
                    ctxuo[b, :, :, 0:ps], st["bs"]["ctxu"][:, :, 0:ps]
                )
            sp = psum_s.tile([128, 512], f32, tag="scps", name=f"sc_{u}")
            eh = spool.tile([128, 512], bf16, tag="eh", name=f"eh_{u}")
            dh = spool.tile([128, 1], f32, tag="dh", name=f"dh_{u}")
            for g in range(G):
                for q in range(nq):
                    nc.tensor.matmul(
                        sp[32 * q : 32 * q + 1, 0:cw],
                        lhsT=w2_t[:, g : g + 1],
                        rhs=st["hs"][g][:, q * cw : (q + 1) * cw],
                        start=(g == 0),
                        stop=(g == G - 1),
                        tile_position=(0, 32 * q) if coltile else None,
                    )
            slot0 = len(st["bs"]["slots"])
            for q in range(nq):
                slot = len(st["bs"]["slots"])
                nc.scalar.activation(
                    eh[32 * q : 32 * q + 1, 0:cw],
                    sp[32 * q : 32 * q + 1, 0:cw],
                    AF.Exp,
                    accum_out=dh[32 * q : 32 * q + 1, :],
                )
                if q == 0:
                    exp_ap = eh[0:1, 0:cw]
                else:
                    # partition_broadcast ucode only reads partition 0; hop
                    # q=1's exp row down via a tiny SBUF->SBUF DMA
                    eh0 = spool.tile([1, 512], bf16, tag="eh0", name=f"eh0_{u}")
                    nc.scalar.dma_start(eh0[0:1, 0:cw], eh[32:33, 0:cw])
                    exp_ap = eh0[0:1, 0:cw]
                emit_ctx_part(u, lo + q * cw, cw, slot, exp_ap)
            final = lo + W == S and b == BLOC - 1
            eng = nc.sync if final else nc.gpsimd
            eng.dma_start(
                expo[b : b + 1, lo : lo + W].rearrange("o (q s) -> (o q) s", q=nq),
                eh[0 : 32 * (nq - 1) + 1 : 32, 0:cw],
            )
            eng.dma_start(
                den4o[b : b + 1, slot0 : slot0 + nq].rearrange("o q -> q o"),
                dh[0 : 32 * (nq - 1) + 1 : 32, :],
            )
            ns = len(st["bs"]["slots"])
            if final:
                # ship this unit's freshly written ctx slots (the earlier ones
                # were staged at the top of this function)
                nc.sync.dma_start(
                    ctxuo[b, :, :, ns - nq : ns], st["bs"]["ctxu"][:, :, ns - nq : ns]
                )
            elif lo + W == S:
                nc.gpsimd.dma_start(
                    ctxuo[b, :, :, 0:ns], st["bs"]["ctxu"][:, :, 0:ns]
                )

        n_units = len(units)
        for u in range(n_units):
            hook = (lambda uu=u: emit_scores_ctx(uu - 1)) if u >= 1 else None
            emit_main(u, mid_hook=hook)
        emit_scores_ctx(n_units - 1)

    nc.compile()
    return nc


def _get_nc(reps=1):
    key = f"nc{reps}"
    if key not in _CACHE:
        _CACHE[key] = _build(reps)
    return _CACHE[key]


def _prep_inputs(x, W1, b1, w2):
    bf = ml_dtypes.bfloat16
    xt_all = np.ascontiguousarray(x.transpose(0, 2, 1)).astype(bf)  # [B, D, S]
    w1t = np.asarray(W1).T.reshape(C, 128, G, 128).transpose(1, 0, 2, 3)
    w1t = np.ascontiguousarray(w1t).astype(bf)  # [p, c, g, e]
    b1r = np.ascontiguousarray(np.asarray(b1).reshape(G, 128).T).astype(np.float32)
    w2r = np.ascontiguousarray(np.asarray(w2).reshape(G, 128).T).astype(bf)
    return [
        {
            "xt": np.ascontiguousarray(xt_all[c * BLOC : (c + 1) * BLOC]),
            "w1t": w1t,
            "b1r": b1r,
            "w2r": w2r,
        }
        for c in range(NCORES)
    ]


def run(x, W1, b1, w2, b2, **spmd_kwargs):
    """Run on hardware; returns (ctx, w, BassKernelResults)."""
    from concourse.bass_utils import run_bass_kernel_spmd

    nc = _get_nc()
    in_maps = _prep_inputs(np.asarray(x), W1, b1, w2)
    res = run_bass_kernel_spmd(nc, in_maps, core_ids=list(range(NCORES)), **spmd_kwargs)
    exp = np.concatenate(
        [res.results[c]["expo"].astype(np.float32) for c in range(NCORES)], axis=0
    )  # [B, S]
    den = np.concatenate(
        [res.results[c]["den4o"][:, :4].sum(axis=1) for c in range(NCORES)], axis=0
    )  # [B]
    ctxu = np.concatenate(
        [res.results[c]["ctxuo"] for c in range(NCORES)], axis=0
    )  # [B, 128, C, NSLOT]
    w = exp / den[:, None]
    # every batch writes exactly 4 score-chunk slots; ignore the spare
    su = ctxu[:, :, :, :4].sum(axis=3)
    ctx = su.transpose(0, 2, 1).reshape(B, D) / den[:, None]
    return ctx.astype(np.float32), w[..., None].astype(np.float32), res


def kernel(x, W1, b1, w2, b2):
    ctx, w, _ = run(x, W1, b1, w2, b2)
    return ctx, w
